# revision 31
# baseline (speedup 1.0000x reference)
"""CrossAttentionFusion kernel for Trainium2 (8 NeuronCores, Bass/Tile).

Computation (matches the reference nn.Module):
  image_proj = relu(BN(1x1conv(image_features, image_w)))   # (B,128,H,W)
  lidar_proj = relu(BN(1x1conv(lidar_features, lidar_w)))   # (B,128,H,W)
  per (batch, 2048-pixel chunk): q = image_proj, k = v = lidar_proj
  attn_out = softmax(q k^T / sqrt(128)) @ k
  out = w0 * image_proj + w1 * attn_out,  w = softmax(modality_weights)

Sharding: the 16 independent (batch, chunk) attention problems are
distributed 2-per-core across 8 cores; each core also computes the
projections for its own pixels.  Host gathers the 8 outputs.

Per-core kernel layout notes (v2, bf16):
  - All matmul operands are bf16 (fp32 PSUM accumulate): halves the
    LDWEIGHTS cost, halves DMA, and avoids the fp32r power throttle.
  - w0 is folded into the image BN affine, w1 into the lidar BN affine;
    the exp scale compensates with 1/(w0*w1*sqrt(C)).  The final combine
    is then res = po/denom + qT.
  - Scores are k-major: sT[kpix, q] via matmul with both operands
    channel-major.  exp() on ACT writes bf16 ET.
  - The softmax denominator accumulates S += ET_i on the (otherwise
    idle) Pool engine in f32r, then ones^T @ S broadcast-sums across
    partitions on the PE.
  - AV uses transposed-K bf16 tiles: po[c, q] += Kpix_j^T @ ET_j.
  - Output is written bf16 and cast to fp32 on the host.
"""

import math
import os
import sys
from contextlib import ExitStack

import numpy as np

sys.path.insert(0, "/opt/trn_rl_repo")

import concourse.bass as bass  # noqa: E402
import concourse.tile as tile  # noqa: E402
from concourse import bacc, mybir  # noqa: E402
from concourse.bass import ds, ts  # noqa: E402
from concourse.bass_utils import run_bass_kernel_spmd  # noqa: E402

F32 = mybir.dt.float32
F32R = mybir.dt.float32r
BF16 = mybir.dt.bfloat16

B, CL, CI, CO = 2, 256, 512, 128
H = W = 128
P = H * W                    # 16384 pixels per batch
CHUNK = 2048                 # attention chunk (pixels)
NCH = P // CHUNK             # 8 chunks per batch
NCORES = 8
UPC = (B * NCH) // NCORES    # units (b,chunk) per core = 2
EPS = 1e-5
QB = 1024                    # q-block width (2 matmul halves of 512)
NQB = CHUNK // QB            # 2
KSL = CHUNK // 128           # 16 k-pixel slices per chunk
NCI_IMG = CI // 128          # 4 contraction slices for image proj
NCI_LID = CL // 128          # 2 for lidar proj

_PROGRAM = None              # compiled Bass program, built once per process
LAST_RESULTS = None          # BassKernelResults of the last kernel() call


def _build_program():
    nc = bacc.Bacc("TRN2", target_bir_lowering=False, debug=False,
                   num_devices=NCORES)

    # Per-core DRAM inputs (pre-sharded on host, bf16).
    ximg = nc.dram_tensor("ximg", [UPC, NCI_IMG, 128, CHUNK], BF16,
                          kind="ExternalInput").ap()
    xlid = nc.dram_tensor("xlid", [UPC, NCI_LID, 128, CHUNK], BF16,
                          kind="ExternalInput").ap()
    wimg = nc.dram_tensor("wimg", [128, NCI_IMG * CO], BF16,
                          kind="ExternalInput").ap()
    wlid = nc.dram_tensor("wlid", [128, NCI_LID * CO], BF16,
                          kind="ExternalInput").ap()
    scal = nc.dram_tensor("scal", [128, 5], F32, kind="ExternalInput").ap()
    idon = nc.dram_tensor("idon", [128, 256], BF16, kind="ExternalInput").ap()
    y = nc.dram_tensor("y", [UPC, CO, CHUNK], BF16, kind="ExternalOutput").ap()

    with tile.TileContext(nc) as tc, ExitStack() as ctx:
        const = ctx.enter_context(tc.tile_pool(name="const", bufs=1))
        xi_pool = ctx.enter_context(tc.tile_pool(name="xi", bufs=UPC * NCI_IMG))
        xl_pool = ctx.enter_context(tc.tile_pool(name="xl", bufs=UPC * NCI_LID))
        proj_pool = ctx.enter_context(tc.tile_pool(name="proj", bufs=4))
        kp_pool = ctx.enter_context(tc.tile_pool(name="kp", bufs=4))
        et_pool = ctx.enter_context(tc.tile_pool(name="et", bufs=8))
        s_pool = ctx.enter_context(tc.tile_pool(name="s", bufs=4))
        misc_pool = ctx.enter_context(tc.tile_pool(name="misc", bufs=4))
        res_pool = ctx.enter_context(tc.tile_pool(name="res", bufs=2))
        # PSUM: mm 2x[128,1024]f32 (4 banks) + av 2x[128,1024]f32 (4 banks).
        # Transposes (bf16) and the denominator matmul borrow mm slots.
        mm_psum = ctx.enter_context(tc.tile_pool(name="mmps", bufs=2, space="PSUM"))
        av_psum = ctx.enter_context(tc.tile_pool(name="avps", bufs=2, space="PSUM"))

        # constants, packed into 4 transfers on the gpsimd DMA queue (in
        # parallel with the input stream on the sync queue)
        wimg_t = const.tile([128, NCI_IMG * CO], BF16)
        nc.gpsimd.dma_start(wimg_t[:], wimg)
        scal_t = const.tile([128, 5], F32)
        nc.gpsimd.dma_start(scal_t[:], scal)
        wlid_t = const.tile([128, NCI_LID * CO], BF16)
        nc.gpsimd.dma_start(wlid_t[:], wlid)
        idon_t = const.tile([128, 256], BF16)
        nc.gpsimd.dma_start(idon_t[:], idon)
        img_s = scal_t[:, ds(0, 1)]
        img_b = scal_t[:, ds(1, 1)]
        lid_s = scal_t[:, ds(2, 1)]
        lid_b = scal_t[:, ds(3, 1)]
        esc = scal_t[:, ds(4, 1)]
        ident_t = idon_t[:, ds(0, 128)]
        ones_t = idon_t[:, ds(128, 128)]

        # ---- prefetch ALL unit inputs up front so the in-order DMA queue
        # never serializes later units' loads behind the y output DMA ----
        xi_u, xl_u = [], []
        for u in range(UPC):
            xi = [xi_pool.tile([128, CHUNK], BF16, name=f"xi_{u}_{ci}", tag="xi")
                  for ci in range(NCI_IMG)]
            xl = [xl_pool.tile([128, CHUNK], BF16, name=f"xl_{u}_{ci}", tag="xl")
                  for ci in range(NCI_LID)]
            if u == 0:
                # ordered to match unit 0's emission: img qb0, lid qb0,
                # lid qb1, then img qb1 (deferred past the transposes)
                for ci in range(NCI_IMG):
                    nc.sync.dma_start(xi[ci][:, ts(0, QB)],
                                      ximg[u, ci, :, ts(0, QB)])
                for hh in range(2):
                    for ci in range(NCI_LID):
                        nc.sync.dma_start(xl[ci][:, ts(hh, QB)],
                                          xlid[u, ci, :, ts(hh, QB)])
                for ci in range(NCI_IMG):
                    nc.sync.dma_start(xi[ci][:, ts(1, QB)],
                                      ximg[u, ci, :, ts(1, QB)])
            else:
                for ci in range(NCI_IMG):
                    for hh in range(2):
                        nc.sync.dma_start(xi[ci][:, ts(hh, QB)],
                                          ximg[u, ci, :, ts(hh, QB)])
                for ci in range(NCI_LID):
                    for hh in range(2):
                        nc.sync.dma_start(xl[ci][:, ts(hh, QB)],
                                          xlid[u, ci, :, ts(hh, QB)])
            xi_u.append(xi)
            xl_u.append(xl)

        LOOKAHEAD = 4  # AV matmuls lag scores so the in-order PE queue
        #                never stalls waiting on ACT-engine exp

        def emit_proj_img(u, qb, qT, pool=None, off_act=False):
            """Image projection for one q-block (8 matmuls + relu)."""
            xi = xi_u[u]
            ps = (pool or mm_psum).tile([128, QB], F32,
                                        name=f"psi_{u}_{qb}",
                                        tag="mm" if pool is None else "av")
            for h in range(QB // 512):
                for ci in range(NCI_IMG):
                    nc.tensor.matmul(ps[:, ts(h, 512)], wimg_t[:, ts(ci, CO)],
                                     xi[ci][:, ds(qb * QB + h * 512, 512)],
                                     start=(ci == 0), stop=(ci == NCI_IMG - 1))
            if off_act:
                # keep ACT exp-only: affine on DVE, relu on Pool (bf16 2x)
                aff = misc_pool.tile([128, QB], BF16, name=f"afi_{u}_{qb}",
                                     tag="aff")
                nc.vector.tensor_scalar(aff[:], ps[:], img_s, img_b,
                                        op0=mybir.AluOpType.mult,
                                        op1=mybir.AluOpType.add)
                nc.gpsimd.tensor_scalar_max(qT[:, ts(qb, QB)], aff[:], 0.0)
            else:
                nc.scalar.activation(qT[:, ts(qb, QB)], ps[:],
                                     mybir.ActivationFunctionType.Relu,
                                     bias=img_b, scale=img_s)

        def emit_proj_lid(u, qb, kT, pool=None, off_act=False):
            """Lidar projection for one q-block (4 matmuls + relu)."""
            xl = xl_u[u]
            ps2 = (pool or mm_psum).tile([128, QB], F32,
                                         name=f"psl_{u}_{qb}",
                                         tag="mm" if pool is None else "av")
            for h in range(QB // 512):
                for ci in range(NCI_LID):
                    nc.tensor.matmul(ps2[:, ts(h, 512)], wlid_t[:, ts(ci, CO)],
                                     xl[ci][:, ds(qb * QB + h * 512, 512)],
                                     start=(ci == 0), stop=(ci == NCI_LID - 1))
            if off_act:
                aff = misc_pool.tile([128, QB], BF16, name=f"afl_{u}_{qb}",
                                     tag="aff")
                nc.vector.tensor_scalar(aff[:], ps2[:], lid_s, lid_b,
                                        op0=mybir.AluOpType.mult,
                                        op1=mybir.AluOpType.add)
                nc.gpsimd.tensor_scalar_max(kT[:, ts(qb, QB)], aff[:], 0.0)
            else:
                nc.scalar.activation(kT[:, ts(qb, QB)], ps2[:],
                                     mybir.ActivationFunctionType.Relu,
                                     bias=lid_b, scale=lid_s)

        def emit_transpose(u, g, kT, kpw):
            """Transpose 8 k-slices to pixel-major bf16 (one-bank PSUM tile
            borrowed from the mm pool, one wide DVE copy)."""
            pt = mm_psum.tile([128, 8 * 128], BF16, name=f"pt_{u}_{g}", tag="mm")
            for k in range(8):
                nc.tensor.transpose(pt[:, ts(k, 128)],
                                    kT[:, ts(g * 8 + k, 128)], ident_t)
            kpt = kp_pool.tile([128, 8 * 128], BF16, name=f"kp_{u}_{g}", tag="kp")
            nc.vector.tensor_copy(kpt[:], pt[:])
            kpw.append(kpt)

        def make_tail(u, qb, po, SA, SB, qT, res_u, last):
            """Deferred q-block tail: denominator matmul + final combine.
            Emitted a couple of slices into the NEXT q-block so the PE
            never stalls waiting on the DVE S-chains."""
            def tail():
                pl = mm_psum.tile([128, QB], F32, name=f"pl_{u}_{qb}", tag="mm")
                for h in range(QB // 512):
                    nc.tensor.matmul(pl[:, ts(h, 512)], ones_t,
                                     SA[:, ts(h, 512)], start=True, stop=False)
                    nc.tensor.matmul(pl[:, ts(h, 512)], ones_t,
                                     SB[:, ts(h, 512)], start=False, stop=True)
                linv = misc_pool.tile([128, QB], F32, name=f"linv_{u}_{qb}",
                                      tag="linv")
                tmp = misc_pool.tile([128, QB], BF16, name=f"tmp_{u}_{qb}",
                                     tag="tmp")
                for h in range(QB // 512):
                    nc.vector.reciprocal_approx_fast(linv[:, ts(h, 512)],
                                                     pl[:, ts(h, 512)])
                    nc.vector.tensor_mul(tmp[:, ts(h, 512)], po[:, ts(h, 512)],
                                         linv[:, ts(h, 512)])
                    eng = nc.vector if last else nc.gpsimd
                    eng.tensor_add(res_u[:, ds(qb * QB + h * 512, 512)],
                                   tmp[:, ts(h, 512)],
                                   qT[:, ds(qb * QB + h * 512, 512)])
                    nc.sync.dma_start(y[u, :, ds(qb * QB + h * 512, 512)],
                                      res_u[:, ds(qb * QB + h * 512, 512)])
            return tail

        # software pipeline across q-blocks and units: each q-block's tail
        # and the next unit's projections are emitted inside the following
        # q-block's slice loop, keeping the PE queue dense at boundaries.
        proj_state = {}

        def emit_proj_all(u):
            # img qb1 goes last, after the transposes: its input DMA halves
            # arrive last and nothing before the qb1 scores needs it
            qT = proj_pool.tile([128, CHUNK], BF16, name=f"qT_{u}", tag="qT")
            kT = proj_pool.tile([128, CHUNK], BF16, name=f"kT_{u}", tag="kT")
            kpw = []
            emit_proj_img(u, 0, qT)
            emit_proj_lid(u, 0, kT)
            emit_proj_lid(u, 1, kT)
            emit_transpose(u, 0, kT, kpw)
            emit_transpose(u, 1, kT, kpw)
            emit_proj_img(u, 1, qT)
            proj_state[u] = (qT, kT, kpw)

        emit_proj_all(0)
        pending_tail = None
        for u in range(UPC):
            qT, kT, kpw = proj_state[u]
            res_u = res_pool.tile([128, CHUNK], BF16, name=f"res_{u}", tag="res")
            for qb in range(NQB):
                boundary = (u < UPC - 1 and qb == NQB - 1)
                po = av_psum.tile([128, QB], F32, name=f"po_{u}_{qb}", tag="av")
                # softmax denominator: two bf16 partial sums (even/odd k-
                # slices), both on DVE where all-bf16 SBUF ops hit the 2x/4x
                # perf modes; recombined in fp32 by the ones^T matmul
                SA = s_pool.tile([128, QB], BF16, name=f"SA_{u}_{qb}", tag="S")
                SB = s_pool.tile([128, QB], BF16, name=f"SB_{u}_{qb}", tag="S")
                lastq = (u == UPC - 1 and qb == NQB - 1)
                ets = [None] * KSL
                for i in range(KSL + LOOKAHEAD):
                    if i < KSL:
                        ps = mm_psum.tile([128, QB], F32,
                                          name=f"pss_{u}_{qb}_{i}", tag="mm")
                        et = et_pool.tile([128, QB], BF16,
                                          name=f"et_{u}_{qb}_{i}", tag="et")
                        S = SA if i % 2 == 0 else SB
                        if lastq and i == KSL - 1:
                            # final slice of the kernel: process per-half so
                            # the exp -> S-add -> denominator -> finals chain
                            # pipelines instead of serializing at full width
                            for h in range(QB // 512):
                                nc.tensor.matmul(ps[:, ts(h, 512)],
                                                 kT[:, ts(i, 128)],
                                                 qT[:, ds(qb * QB + h * 512, 512)],
                                                 start=True, stop=True)
                                nc.scalar.activation(
                                    et[:, ts(h, 512)], ps[:, ts(h, 512)],
                                    mybir.ActivationFunctionType.Exp, scale=esc)
                                nc.vector.tensor_add(S[:, ts(h, 512)],
                                                     S[:, ts(h, 512)],
                                                     et[:, ts(h, 512)])
                            ets[i] = et
                        else:
                            for h in range(QB // 512):
                                nc.tensor.matmul(ps[:, ts(h, 512)],
                                                 kT[:, ts(i, 128)],
                                                 qT[:, ds(qb * QB + h * 512, 512)],
                                                 start=True, stop=True)
                            nc.scalar.activation(et[:], ps[:],
                                                 mybir.ActivationFunctionType.Exp,
                                                 scale=esc)
                            ets[i] = et
                            if i < 2:
                                nc.vector.tensor_copy(S[:], et[:])
                            else:
                                nc.vector.tensor_add(S[:], S[:], et[:])
                    j = i - LOOKAHEAD
                    if j >= 0:
                        kslice = kpw[j // 8][:, ts(j % 8, 128)]
                        for h in range(QB // 512):
                            nc.tensor.matmul(po[:, ts(h, 512)], kslice,
                                             ets[j][:, ts(h, 512)],
                                             start=(j == 0), stop=(j == KSL - 1))
                    if i == 2 and pending_tail is not None:
                        pending_tail()
                        pending_tail = None
                    if i == KSL - 1 and boundary:
                        nqT = proj_pool.tile([128, CHUNK], BF16,
                                             name=f"qT_{u + 1}", tag="qT")
                        nkT = proj_pool.tile([128, CHUNK], BF16,
                                             name=f"kT_{u + 1}", tag="kT")
                        emit_proj_lid(u + 1, 0, nkT)
                last = (u == UPC - 1 and qb == NQB - 1)
                tail = make_tail(u, qb, po, SA, SB, qT, res_u, last)
                if last:
                    tail()
                elif boundary:
                    # unit boundary: next unit's projections slot between
                    # this q-block's AV tail and its denominator.  Lidar
                    # first — the transposes and the next attention's scores
                    # depend only on kT, so its relu chain is the critical
                    # path; image projections fill the PE while it drains.
                    nkpw = []
                    tail()
                    emit_proj_lid(u + 1, 1, nkT)
                    emit_transpose(u + 1, 0, nkT, nkpw)
                    emit_proj_img(u + 1, 0, nqT)
                    emit_transpose(u + 1, 1, nkT, nkpw)
                    emit_proj_img(u + 1, 1, nqT)
                    proj_state[u + 1] = (nqT, nkT, nkpw)
                else:
                    pending_tail = tail

    nc.compile()
    return nc


def _to_bf16(a):
    """Round-to-nearest-even fp32 -> bf16, returned as ml_dtypes.bfloat16."""
    import ml_dtypes
    return np.asarray(a, np.float32).astype(ml_dtypes.bfloat16)


def _shard_inputs(inputs):
    """Build the 8 per-core input maps from the full input dict."""
    import ml_dtypes
    mw = np.asarray(inputs["modality_weights"], np.float64)
    e = np.exp(mw - mw.max())
    w = (e / e.sum()).astype(np.float64)
    w0, w1 = float(w[0]), float(w[1])

    def bn_fold(gamma, beta, mean, var, mul):
        g = np.asarray(gamma, np.float64)
        b = np.asarray(beta, np.float64)
        m = np.asarray(mean, np.float64)
        v = np.asarray(var, np.float64)
        scale = g / np.sqrt(v + EPS) * mul
        bias = (b - m * g / np.sqrt(v + EPS)) * mul
        return (scale.astype(np.float32).reshape(CO, 1),
                bias.astype(np.float32).reshape(CO, 1))

    i_s, i_b = bn_fold(inputs["image_gamma"], inputs["image_beta"],
                       inputs["image_mean"], inputs["image_var"], w0)
    l_s, l_b = bn_fold(inputs["lidar_gamma"], inputs["lidar_beta"],
                       inputs["lidar_mean"], inputs["lidar_var"], w1)

    # weight slices, pre-transposed for lhsT ([cin_slice, cout]) and packed
    # ci-major along columns: [128, NCI*CO], bf16
    wi = _to_bf16(np.ascontiguousarray(
        np.asarray(inputs["image_w"], np.float32).T.reshape(NCI_IMG, 128, CO)
        .transpose(1, 0, 2).reshape(128, NCI_IMG * CO)))
    wl = _to_bf16(np.ascontiguousarray(
        np.asarray(inputs["lidar_w"], np.float32).T.reshape(NCI_LID, 128, CO)
        .transpose(1, 0, 2).reshape(128, NCI_LID * CO)))

    esc = np.full((128, 1), 1.0 / (w0 * w1 * math.sqrt(CO)), np.float32)
    scal = np.concatenate([i_s, i_b, l_s, l_b, esc], axis=1)
    idon = np.concatenate([np.eye(128, dtype=ml_dtypes.bfloat16),
                           np.ones((128, 128), ml_dtypes.bfloat16)], axis=1)

    # full features reshaped to (B, nchunks, C, 2048), cast once to bf16
    img = _to_bf16(np.asarray(inputs["image_features"], np.float32)
                   ).reshape(B, CI, NCH, CHUNK)
    lid = _to_bf16(np.asarray(inputs["lidar_features"], np.float32)
                   ).reshape(B, CL, NCH, CHUNK)

    in_maps = []
    for core in range(NCORES):
        ximg = np.empty((UPC, NCI_IMG, 128, CHUNK), ml_dtypes.bfloat16)
        xlid = np.empty((UPC, NCI_LID, 128, CHUNK), ml_dtypes.bfloat16)
        for ul in range(UPC):
            un = core * UPC + ul
            b, c = un // NCH, un % NCH
            ximg[ul] = img[b, :, c, :].reshape(NCI_IMG, 128, CHUNK)
            xlid[ul] = lid[b, :, c, :].reshape(NCI_LID, 128, CHUNK)
        in_maps.append({
            "ximg": ximg, "xlid": xlid, "wimg": wi, "wlid": wl,
            "scal": scal, "idon": idon,
        })
    return in_maps


def kernel(**inputs) -> np.ndarray:
    global _PROGRAM, LAST_RESULTS
    if _PROGRAM is None:
        _PROGRAM = _build_program()
    nc = _PROGRAM

    in_maps = _shard_inputs(inputs)
    trace = os.environ.get("BASS_KERNEL_TRACE", "0") == "1"
    tmpdir = os.environ.get("BASS_KERNEL_TRACE_DIR") or None
    if tmpdir:
        os.makedirs(tmpdir, exist_ok=True)
    results = run_bass_kernel_spmd(nc, in_maps, core_ids=list(range(NCORES)),
                                   trace=trace, tmpdir=tmpdir)
    LAST_RESULTS = results

    out = np.empty((B, CO, H, W), np.float32)
    outv = out.reshape(B, CO, NCH, CHUNK)
    for core in range(NCORES):
        yc = results.results[core]["y"]
        for ul in range(UPC):
            un = core * UPC + ul
            b, c = un // NCH, un % NCH
            outv[b, :, c, :] = np.asarray(yc[ul], np.float32)
    return out


if __name__ == "__main__":
    rng = np.random.default_rng(0)
    inputs = {
        "lidar_features": rng.standard_normal((B, CL, H, W), np.float32),
        "image_features": rng.standard_normal((B, CI, H, W), np.float32),
        "lidar_w": rng.standard_normal((CO, CL), np.float32) * np.sqrt(2.0 / CO),
        "lidar_gamma": np.ones(CO, np.float32),
        "lidar_beta": np.zeros(CO, np.float32),
        "lidar_mean": rng.standard_normal(CO).astype(np.float32) * 0.1,
        "lidar_var": rng.uniform(0.5, 1.5, CO).astype(np.float32),
        "image_w": rng.standard_normal((CO, CI), np.float32) * np.sqrt(2.0 / CO),
        "image_gamma": np.ones(CO, np.float32),
        "image_beta": np.zeros(CO, np.float32),
        "image_mean": rng.standard_normal(CO).astype(np.float32) * 0.1,
        "image_var": rng.uniform(0.5, 1.5, CO).astype(np.float32),
        "modality_weights": np.ones(2, np.float32),
    }
    out = kernel(**inputs)
    print("kernel out:", out.shape, out.dtype, float(np.abs(out).mean()))
